# revision 29
# baseline (speedup 1.0000x reference)
"""Trainium2 Bass kernel for nn_MultiHeadAttention (B=2, S=2048, D=1024, H=16).

Sharding: batch*heads across 8 cores -> each core handles one batch element's
4 heads (core c: b = c//4, heads h0 = (c%4)*4 .. h0+4).

Key idea: the padding mask kills ~half the keys, and in the transposed
"scoresT" layout [key, query] masked keys are whole partitions. The host
gathers each head's unmasked key positions (padded to KT tiles of 128), so
the device only computes scores/exp/ctx over ~9 instead of 16 key tiles --
numerically identical to masking (masked terms contribute exactly 0).

Per-core device program (fp16 matmuls, f32 PSUM):
  1. Q projection from pre-transposed activations xT [1024, 2048] into two
     zero-padded transposed tiles (Qt0/Qt1) so score matmuls run with K=128.
  2. Per-head K/V projection from gathered activations xg[h] [1024, KT*128]:
     K transposed via a pair-weight "discard" trick, V natural with a ones
     column appended (softmax-sum trick).
  3. Attention per head pair in scoresT layout: PE scores -> ScalarE exp
     (additive pad-mask via per-partition bias) -> PE ctx accumulation
     [65, q]: row 64 = softmax sums. Software-pipelined so ScalarE (the
     bottleneck) never waits; normalization deferred into the next block.
  4. Output projection ctxT pairs x W_out rows -> fp16 partial [2048, 1024].
Host sums the 4 partial outputs per batch element and adds b_out.
"""

import math
import os

import numpy as np

# Tile's fine-grained (subtile) dependency tracker misses some of this
# kernel's partition-sliced producer->consumer edges (verified empirically:
# per-core divergent results with it on, bit-identical and correct with it
# off). Coarse tile-level deps cost ~8us here and are always safe.
os.environ.setdefault("BY_DEFAULT_DISABLE_SUBTILE_DEPS", "1")

N_HEADS = 16
DIM = 1024
DIM_PER_HEAD = 64
B = 2
S = 2048
SCALE = math.sqrt(DIM_PER_HEAD)
N_CORES = 8
HEADS_PER_CORE = 4

_cache = {}


def _build_program(KT):
    import concourse.tile as tile
    from concourse import bacc, mybir

    f32 = mybir.dt.float32
    fp16 = mybir.dt.float16
    Exp = mybir.ActivationFunctionType.Exp
    SK = KT * 128  # gathered (padded) key count per head

    nc = bacc.Bacc("TRN2", target_bir_lowering=False, debug=False,
                   num_devices=N_CORES)

    xT = nc.dram_tensor("xT", [DIM, S], fp16, kind="ExternalInput").ap()
    xg = nc.dram_tensor("xg", [4, DIM, SK], fp16, kind="ExternalInput").ap()
    Wq = nc.dram_tensor("Wq", [DIM, 256], fp16, kind="ExternalInput").ap()
    Wk = nc.dram_tensor("Wk", [DIM, 256], fp16, kind="ExternalInput").ap()
    Wv = nc.dram_tensor("Wv", [DIM, 256], fp16, kind="ExternalInput").ap()
    Wo = nc.dram_tensor("Wo", [256, DIM], fp16, kind="ExternalInput").ap()
    bqk = nc.dram_tensor("bqk", [128, 4], f32, kind="ExternalInput").ap()
    bv = nc.dram_tensor("bv", [1, 256], fp16, kind="ExternalInput").ap()
    maskT = nc.dram_tensor("maskT", [128, 4 * KT], f32,
                           kind="ExternalInput").ap()
    ones_d = nc.dram_tensor("ones_d", [1, 512], fp16,
                            kind="ExternalInput").ap()
    ones_bf = nc.dram_tensor("ones_bf", [128, 64], fp16,
                             kind="ExternalInput").ap()
    zeros_d = nc.dram_tensor("zeros_d", [64, 4096], fp16,
                             kind="ExternalInput").ap()
    out_d = nc.dram_tensor("out", [S, DIM], fp16, kind="ExternalOutput").ap()

    with tile.TileContext(nc) as tc:
        with tc.tile_pool(name="const", bufs=1) as cpool, \
             tc.tile_pool(name="wpool", bufs=1) as wpool, \
             tc.tile_pool(name="qkv", bufs=1) as qkvp, \
             tc.tile_pool(name="xsub", bufs=33) as xsub, \
             tc.tile_pool(name="xgp", bufs=33) as xgp, \
             tc.tile_pool(name="expp", bufs=5) as expp, \
             tc.tile_pool(name="ctxu", bufs=2) as ctxu, \
             tc.tile_pool(name="outsb", bufs=4) as outsb, \
             tc.tile_pool(name="rscr", bufs=2) as rscr, \
             tc.tile_pool(name="ps", bufs=2, space="PSUM") as ps:

            # --- inputs needed first ---
            bqk_sb = cpool.tile([128, 4], f32)
            nc.sync.dma_start(bqk_sb[:], bqk[:])
            Wq_sb = wpool.tile([128, 8, 256], fp16)
            nc.sync.dma_start(Wq_sb[:], Wq.rearrange("(c p) j -> p c j", p=128))

            xts = []

            def load_xts(sc):
                for dc in range(8):
                    xt_t = xsub.tile([128, 512], fp16, tag="x",
                                     name=f"x_{sc}_{dc}")
                    nc.sync.dma_start(
                        xt_t[:], xT[dc * 128:(dc + 1) * 128,
                                    sc * 512:(sc + 1) * 512])
                    xts.append(xt_t)

            load_xts(0)
            Wk_sb = wpool.tile([128, 8, 256], fp16)
            nc.sync.dma_start(Wk_sb[:], Wk.rearrange("(c p) j -> p c j", p=128))
            for _sc in range(1, 4):
                load_xts(_sc)

            Qt0_sb = qkvp.tile([128, 2, S], fp16)
            Qt1_sb = qkvp.tile([128, 2, S], fp16)
            nc.sync.dma_start(Qt0_sb[64:128, :, :],
                              zeros_d.rearrange("p (c s) -> p c s", c=2))
            nc.sync.dma_start(Qt1_sb[0:64, :, :],
                              zeros_d.rearrange("p (c s) -> p c s", c=2))
            Kt_g = qkvp.tile([128, 2, SK], fp16)
            Vaug = qkvp.tile([128, 4 * KT * 65], fp16)
            v4 = Vaug.rearrange("p (h k j) -> p h k j", h=4, k=KT)
            ctxT_norm = qkvp.tile([128, 2, S], fp16)

            # ---- Q projection (transposed, zero-padded per head) ----
            for sc in range(4):
                for p in range(2):
                    ps_t = ps.tile([128, 512], f32,
                                   tag="a" if p == 0 else "ctx",
                                   name=f"pq_{sc}_{p}")
                    for dc in range(8):
                        nc.tensor.matmul(
                            ps_t[:], lhsT=Wq_sb[:, dc, p * 128:(p + 1) * 128],
                            rhs=xts[sc * 8 + dc][:],
                            start=(dc == 0), stop=(dc == 7))
                    ssl = slice(sc * 512, (sc + 1) * 512)
                    bias = bqk_sb[:, p: p + 1]
                    nc.vector.tensor_scalar_add(
                        Qt0_sb[0:64, p, ssl], ps_t[0:64, :], bias[0:64, :])
                    nc.vector.tensor_scalar_add(
                        Qt1_sb[64:128, p, ssl], ps_t[64:128, :],
                        bias[64:128, :])

            # --- later inputs (DMAs overlap the compute above) ---
            xg_t = {}
            for hl in range(4):
                for dc in range(8):
                    t = xgp.tile([128, SK], fp16, tag="xg",
                                 name=f"xg_{hl}_{dc}")
                    nc.sync.dma_start(t[:], xg[hl, dc * 128:(dc + 1) * 128, :])
                    xg_t[(hl, dc)] = t
            Wv_sb = wpool.tile([128, 8, 256], fp16)
            nc.sync.dma_start(Wv_sb[:], Wv.rearrange("(c p) j -> p c j", p=128))
            bv_sb = cpool.tile([1, 256], fp16)
            nc.sync.dma_start(bv_sb[:], bv[:])
            ones = cpool.tile([1, 512], fp16)
            nc.sync.dma_start(ones[:], ones_d[:])
            nc.sync.dma_start(v4[:, :, :, 64],
                              ones_bf[:, 0:4 * KT].rearrange(
                                  "p (h k) -> p h k", h=4))
            maskT_sb = cpool.tile([128, 4 * KT], f32)
            nc.sync.dma_start(maskT_sb[:], maskT[:])
            Wo_sb = wpool.tile([128, 2, 1024], fp16)
            nc.sync.dma_start(Wo_sb[:], Wo.rearrange("(c p) e -> p c e", p=128))

            # ---- K projection from gathered keys (pair-discard trick) ----
            nchunks = []
            n0 = 0
            while n0 < SK:
                nn = min(512, SK - n0)
                nchunks.append((n0, nn))
                n0 += nn
            for hl in range(4):
                p, hp = hl // 2, hl % 2
                for ci, (c0, nn) in enumerate(nchunks):
                    ps_t = ps.tile([128, 512], f32,
                                   tag="a" if (hl + ci) % 2 == 0 else "ctx",
                                   name=f"pk_{hl}_{ci}")
                    for dc in range(8):
                        nc.tensor.matmul(
                            ps_t[:, 0:nn],
                            lhsT=Wk_sb[:, dc, p * 128:(p + 1) * 128],
                            rhs=xg_t[(hl, dc)][:, c0:c0 + nn],
                            start=(dc == 0), stop=(dc == 7))
                    bias = bqk_sb[:, 2 + p: 3 + p]
                    nc.vector.tensor_scalar_add(
                        Kt_g[hp * 64:(hp + 1) * 64, p, c0:c0 + nn],
                        ps_t[hp * 64:(hp + 1) * 64, 0:nn],
                        bias[hp * 64:(hp + 1) * 64, :])

            # ---- V projection from gathered keys (natural, per head) ----
            for hl in range(4):
                for kt in range(KT):
                    pv_t = ps.tile([128, 64], f32,
                                   tag="a" if kt % 2 == 0 else "ctx",
                                   name=f"pv_{hl}_{kt}")
                    for dc in range(8):
                        nc.tensor.matmul(
                            pv_t[:],
                            lhsT=xg_t[(hl, dc)][:, kt * 128:(kt + 1) * 128],
                            rhs=Wv_sb[:, dc, hl * 64:(hl + 1) * 64],
                            start=(dc == 0), stop=False)
                    nc.tensor.matmul(pv_t[:], lhsT=ones[0:1, 0:128],
                                     rhs=bv_sb[0:1, hl * 64:(hl + 1) * 64],
                                     start=False, stop=True)
                    nc.vector.tensor_copy(v4[:, hl, kt, 0:64], pv_t[:])

            # ---- output projection helper (also used to keep the PE warm
            # through the final attention drain) ----
            def emit_outproj(qt):
                tag = "a" if (qt < 2 or qt % 2 == 0) else "ctx"
                po = ps.tile([128, 1024], f32, tag=tag, name=f"po_{qt}")
                for p_ in range(2):
                    for ec in range(2):
                        nc.tensor.matmul(
                            po[:, ec * 512:(ec + 1) * 512],
                            lhsT=ctxT_norm[:, p_, qt * 128:(qt + 1) * 128],
                            rhs=Wo_sb[:, p_, ec * 512:(ec + 1) * 512],
                            start=(p_ == 0), stop=(p_ == 1))
                ob = outsb.tile([128, 1024], fp16, tag="ob", name=f"ob_{qt}")
                if qt % 2 == 0:
                    nc.vector.tensor_copy(ob[:], po[:])
                else:
                    nc.scalar.copy(ob[:], po[:])
                nc.sync.dma_start(out_d[qt * 128:(qt + 1) * 128, :], ob[:])

            # ---- attention, one head PAIR at a time ----
            # scoresT layout [key, query]; score matmuls run at K=128 against
            # zero-padded Qt tiles with a shared stationary operand. q is
            # processed in halves of 1024 so both heads' ctx accumulators fit
            # in PSUM. ctx matmuls lag one step behind exp so the PE never
            # stalls on ScalarE; normalization is deferred into the next
            # block's loop.
            def make_norm_steps(p_, half_, ctxUs_, rss_):
                box = {}

                def step(j):
                    hp_, qc = j // 2, j % 2
                    if hp_ == 1 and "t" not in box:
                        box["t"] = ctxu.tile([64, 1024], fp16, tag="cn",
                                             bufs=2, name=f"ctxN_{p_}_{half_}")
                    pb_t = ps.tile([128, 512], f32, tag="a",
                                   name=f"pb_{p_}_{half_}_{j}")
                    nc.tensor.matmul(pb_t[:], lhsT=ones[0:1, 0:128],
                                     rhs=rss_[hp_][0:1, qc * 512:(qc + 1) * 512],
                                     start=True, stop=True)
                    if hp_ == 0:
                        tt_out = ctxT_norm[0:64, p_,
                                           half_ * 1024 + qc * 512:
                                           half_ * 1024 + (qc + 1) * 512]
                    else:
                        tt_out = box["t"][0:64, qc * 512:(qc + 1) * 512]
                    nc.vector.tensor_mul(
                        tt_out,
                        ctxUs_[hp_][0:64, qc * 512:(qc + 1) * 512],
                        pb_t[0:64, :])
                    if hp_ == 1 and qc == 1:
                        nc.sync.dma_start(
                            ctxT_norm[64:128, p_,
                                      half_ * 1024:(half_ + 1) * 1024],
                            box["t"][0:64, :])

                return [lambda j=j: step(j) for j in range(4)]

            norm_steps = []
            for p in range(2):
                for half in range(2):
                    h0 = p * 2
                    last_block = (p, half) == (1, 1)
                    ctx0 = ps.tile([65, 1024], f32, tag="ctx", bufs=2,
                                   name=f"ctx0_{p}_{half}")
                    ctx1 = ps.tile([65, 1024], f32, tag="ctx", bufs=2,
                                   name=f"ctx1_{p}_{half}")
                    ctxs = (ctx0, ctx1)

                    def emit_scores(kt):
                        s0 = ps.tile([128, 1024], f32, tag="a",
                                     name=f"s0_{p}_{half}_{kt}")
                        s1 = ps.tile([128, 1024], f32, tag="a",
                                     name=f"s1_{p}_{half}_{kt}")
                        lhsT = Kt_g[:, p, kt * 128:(kt + 1) * 128]
                        for qc in range(2):
                            q0 = half * 1024 + qc * 512
                            for s_t, qsrc in ((s0, Qt0_sb), (s1, Qt1_sb)):
                                nc.tensor.matmul(
                                    s_t[:, qc * 512:(qc + 1) * 512],
                                    lhsT=lhsT,
                                    rhs=qsrc[:, p, q0:q0 + 512],
                                    start=True, stop=True)
                        return s0, s1

                    def emit_ctx(kt, ets):
                        for hp in range(2):
                            for qc in range(2):
                                nc.tensor.matmul(
                                    ctxs[hp][:, qc * 512:(qc + 1) * 512],
                                    lhsT=v4[:, h0 + hp, kt, :],
                                    rhs=ets[hp][:, qc * 512:(qc + 1) * 512],
                                    start=(kt == 0), stop=(kt == KT - 1))

                    sc_cur = emit_scores(0)
                    prev_ets = None
                    for kt in range(KT):
                        ets = []
                        for hp in range(2):
                            et = expp.tile([128, 1024], fp16, tag="et",
                                           name=f"et_{p}_{half}_{kt}_{hp}")
                            nc.scalar.activation(
                                et[:], sc_cur[hp][:], Exp,
                                bias=maskT_sb[:, kt * 4 + h0 + hp:
                                              kt * 4 + h0 + hp + 1],
                                scale=1.0)
                            ets.append(et)
                        if prev_ets is not None:
                            emit_ctx(kt - 1, prev_ets)
                        if kt < KT - 1:
                            sc_cur = emit_scores(kt + 1)
                        prev_ets = ets
                        if norm_steps and 3 <= kt <= 6:
                            norm_steps.pop(0)()
                    emit_ctx(KT - 1, prev_ets)
                    if last_block:
                        emit_outproj(0)
                        emit_outproj(1)

                    # drain: move ctx out of PSUM, compute 1/sums per head
                    ctxUs, rss = [], []
                    for hp in range(2):
                        ctxU = ctxu.tile([65, 1024], f32, tag="cu", bufs=4,
                                         name=f"ctxU_{p}_{half}_{hp}")
                        nc.vector.tensor_copy(ctxU[:], ctxs[hp][:])
                        s128 = rscr.tile([128, 8], f32, tag="sm",
                                         name=f"s128_{p}_{half}_{hp}")
                        nc.sync.dma_start(s128[:], ctxU[64:65, :])
                        r128 = rscr.tile([128, 8], fp16, tag="r128",
                                         name=f"r128_{p}_{half}_{hp}")
                        with nc.allow_low_precision(
                                reason="fp16 rounding for matmul rhs"):
                            nc.vector.reciprocal(r128[:], s128[:])
                        rs_t = rscr.tile([1, 1024], fp16, tag="rs",
                                         name=f"rs_{p}_{half}_{hp}")
                        nc.sync.dma_start(rs_t[:], r128[:])
                        ctxUs.append(ctxU)
                        rss.append(rs_t)
                    norm_steps = make_norm_steps(p, half, ctxUs, rss)

            # ---- remaining output projection ----
            # qt 0-7 only needs q<1024 whose normalize is already done; the
            # final norm steps (q 1024:2048 of pair 1) interleave with them.
            for qt in range(2, 8):
                emit_outproj(qt)
                if norm_steps:
                    norm_steps.pop(0)()
            for st_fn in norm_steps:
                st_fn()
            for qt in range(8, 16):
                emit_outproj(qt)

    nc.compile()
    return nc


def get_program(KT=9):
    key = ("nc", KT)
    if key not in _cache:
        _cache[key] = _build_program(KT)
    return _cache[key]


def make_in_maps(query, mask, W_qkv, b_qkv, W_out, b_out):
    query = np.asarray(query, dtype=np.float32)
    mask = np.asarray(mask)
    W_qkv = np.asarray(W_qkv, dtype=np.float32)
    b_qkv = np.asarray(b_qkv, dtype=np.float32)
    W_out = np.asarray(W_out, dtype=np.float32)
    bf = np.float16

    W3 = W_qkv.reshape(DIM, N_HEADS, DIM_PER_HEAD, 3)
    b3 = b_qkv.reshape(N_HEADS, DIM_PER_HEAD, 3)
    m2 = np.asarray(mask)[:, 0, :]  # [32, 2048] True = masked
    KT = max(1, int(np.ceil((~m2).sum(axis=1).max() / 128)))
    SK = KT * 128

    in_maps = []
    for c in range(N_CORES):
        b = c // 4
        h0 = (c % 4) * HEADS_PER_CORE
        hs = slice(h0, h0 + HEADS_PER_CORE)
        Wq_c = np.ascontiguousarray(
            W3[:, hs, :, 0].reshape(DIM, 256) / SCALE).astype(bf)
        Wk_c = np.ascontiguousarray(W3[:, hs, :, 1].reshape(DIM, 256)).astype(bf)
        Wv_c = np.ascontiguousarray(W3[:, hs, :, 2].reshape(DIM, 256)).astype(bf)
        bq_c = (b3[hs, :, 0].reshape(256) / SCALE).astype(np.float32)
        bk_c = b3[hs, :, 1].reshape(256).astype(np.float32)
        bv_c = b3[hs, :, 2].reshape(1, 256).astype(bf)
        bqk_c = np.ascontiguousarray(
            np.stack([bq_c[:128], bq_c[128:], bk_c[:128], bk_c[128:]], axis=1))
        Wo_c = np.ascontiguousarray(
            W_out[h0 * 64:(h0 + 4) * 64, :]).astype(bf)
        xT_c = np.ascontiguousarray(query[b].T).astype(bf)

        xg_c = np.zeros((4, DIM, SK), dtype=bf)
        maskT_c = np.zeros((128, 4 * KT), dtype=np.float32)
        for hl in range(4):
            bh = b * N_HEADS + h0 + hl
            idx = np.nonzero(~m2[bh])[0]
            n = len(idx)
            idx_pad = np.zeros(SK, dtype=np.int64)
            idx_pad[:n] = idx
            xg_c[hl] = xT_c[:, idx_pad]
            padded = np.arange(SK) >= n  # [SK] True = padding slot
            maskT_c[:, hl::4] = np.where(
                padded.reshape(KT, 128).T, np.float32(-30000.0),
                np.float32(0.0))
        in_maps.append({
            "xT": xT_c, "xg": xg_c, "Wq": Wq_c, "Wk": Wk_c, "Wv": Wv_c,
            "Wo": Wo_c, "bqk": bqk_c, "bv": bv_c, "maskT": maskT_c,
            "ones_d": np.ones((1, 512), dtype=bf),
            "ones_bf": np.ones((128, 64), dtype=bf),
            "zeros_d": np.zeros((64, 4096), dtype=bf),
        })
    return in_maps, KT


def gather_outputs(results, b_out):
    b_out = np.asarray(b_out, dtype=np.float32)
    out = np.zeros((B, S, DIM), dtype=np.float32)
    for c in range(N_CORES):
        out[c // 4] += results[c]["out"].astype(np.float32)
    out += b_out[None, None, :]
    return out


def kernel(query, mask, W_qkv, b_qkv, W_out, b_out):
    from concourse.bass_utils import run_bass_kernel_spmd

    in_maps, KT = make_in_maps(query, mask, W_qkv, b_qkv, W_out, b_out)
    nc = get_program(KT)
    res = run_bass_kernel_spmd(nc, in_maps, list(range(N_CORES)))
    return gather_outputs(res.results, b_out)
